# revision 7
# baseline (speedup 1.0000x reference)
import sys

if "/opt/trn_rl_repo" not in sys.path:
    sys.path.insert(0, "/opt/trn_rl_repo")

from contextlib import ExitStack

import numpy as np

import concourse.bacc as bacc
import concourse.tile as tile
from concourse import mybir
from concourse.bass_utils import run_bass_kernel_spmd
from concourse.masks import make_identity

B, H, S, D = 2, 16, 2048, 64
NCORES = 8
PAIRS = (B * H) // NCORES
NT = S // 128
NSEG = S // 512
F32 = mybir.dt.float32
MM_DT = mybir.dt.float32 if __import__("os").environ.get("MM_F32") else mybir.dt.float32r
SCALE = 0.125
NEG = -1e10


def build_nc():
    nc = bacc.Bacc(None)
    q = nc.declare_dram_parameter("q", [PAIRS, S, D], F32, isOutput=False)
    k = nc.declare_dram_parameter("k", [PAIRS, S, D], F32, isOutput=False)
    v = nc.declare_dram_parameter("v", [PAIRS, S, D], F32, isOutput=False)
    out = nc.declare_dram_parameter("out", [PAIRS, S, D], F32, isOutput=True)

    with tile.TileContext(nc) as tc, ExitStack() as ctx:
        consts = ctx.enter_context(tc.tile_pool(name="consts", bufs=1))
        stage = ctx.enter_context(tc.tile_pool(name="stage", bufs=2))
        qtp = ctx.enter_context(tc.tile_pool(name="qt", bufs=2))
        ktp = ctx.enter_context(tc.tile_pool(name="kt", bufs=2))
        vpp = ctx.enter_context(tc.tile_pool(name="vp", bufs=2))
        ptp = ctx.enter_context(tc.tile_pool(name="pt", bufs=2))
        accs_p = ctx.enter_context(tc.tile_pool(name="accs", bufs=2))
        outp = ctx.enter_context(tc.tile_pool(name="outsb", bufs=2))
        smalls = ctx.enter_context(tc.tile_pool(name="smalls", bufs=4))
        ps_scores = ctx.enter_context(
            tc.tile_pool(name="ps_scores", bufs=2, space="PSUM")
        )
        ps_acc = ctx.enter_context(tc.tile_pool(name="ps_acc", bufs=1, space="PSUM"))
        ps_tr = ctx.enter_context(tc.tile_pool(name="ps_tr", bufs=2, space="PSUM"))

        ident = consts.tile([128, 128], F32)
        make_identity(nc, ident)
        trimask = consts.tile([128, 128], F32)
        nc.gpsimd.memset(trimask, 0.0)
        nc.gpsimd.affine_select(
            out=trimask,
            in_=trimask,
            compare_op=mybir.AluOpType.is_ge,
            fill=NEG,
            base=0,
            pattern=[[1, 128]],
            channel_multiplier=-1,
        )

        for p in range(PAIRS):
            qstg = stage.tile([128, NT, D], F32, tag="qstg")
            kstg = stage.tile([128, NT, D], F32, tag="kstg")
            nc.sync.dma_start(
                out=qstg, in_=q[p].rearrange("(t pp) d -> pp t d", pp=128)
            )
            nc.sync.dma_start(
                out=kstg, in_=k[p].rearrange("(t pp) d -> pp t d", pp=128)
            )
            vstg = stage.tile([128, NT, D], F32, tag="vstg")
            nc.sync.dma_start(
                out=vstg, in_=v[p].rearrange("(t pp) d -> pp t d", pp=128)
            )
            vp_t = vpp.tile([128, NT, D + 1], MM_DT, tag="vp")
            nc.vector.tensor_copy(vp_t[:, :, 0:D], vstg)
            nc.vector.memset(vp_t[:, :, D : D + 1], 1.0)

            qt = qtp.tile([64, S], MM_DT, tag="qt")
            kt = ktp.tile([64, S], MM_DT, tag="kt")
            for j in range(NT):
                pst = ps_tr.tile([128, 128], F32, tag="tr")
                nc.tensor.transpose(pst[0:64, 0:128], qstg[:, j, :], ident)
                nc.vector.tensor_copy(qt[:, j * 128 : (j + 1) * 128], pst[0:64, 0:128])
                pst2 = ps_tr.tile([128, 128], F32, tag="tr")
                nc.tensor.transpose(pst2[0:64, 0:128], kstg[:, j, :], ident)
                nc.vector.tensor_copy(kt[:, j * 128 : (j + 1) * 128], pst2[0:64, 0:128])

            acc = ps_acc.tile([128, S], F32, tag="acc")
            for j in range(NT):
                q0 = j * 128
                rem = S - q0
                ptt = ptp.tile([128, S], MM_DT, tag="pt")
                off = 0
                while off < rem:
                    w = min(512, rem - off)
                    ps = ps_scores.tile([128, 512], F32, tag="scores")
                    nc.tensor.matmul(
                        ps[:, 0:w],
                        kt[:, q0 : q0 + 128],
                        qt[:, q0 + off : q0 + off + w],
                        start=True,
                        stop=True,
                    )
                    if off == 0:
                        nc.vector.tensor_add(ps[:, 0:128], ps[:, 0:128], trimask)
                    nc.scalar.activation(
                        ptt[:, off : off + w],
                        ps[:, 0:w],
                        mybir.ActivationFunctionType.Exp,
                        scale=SCALE,
                    )
                    off += w
                for s in range(j // 4, NSEG):
                    gstart = max(512 * s, q0)
                    w2 = 512 * (s + 1) - gstart
                    loc = gstart - q0
                    nc.tensor.matmul(
                        acc[0 : D + 1, gstart : gstart + w2],
                        vp_t[:, j, :],
                        ptt[:, loc : loc + w2],
                        start=(j == 0),
                        stop=(j == 4 * s + 3),
                    )

            accs = accs_p.tile([D + 1, S], F32, tag="accs")
            nc.vector.tensor_copy(accs, acc[0 : D + 1, :])
            osb = outp.tile([128, NT, D], F32, tag="osb")
            for i in range(NT):
                ptr = ps_tr.tile([128, 128], F32, tag="tr")
                nc.tensor.transpose(
                    ptr[0:128, 0 : D + 1],
                    accs[:, i * 128 : (i + 1) * 128],
                    ident[0 : D + 1, 0 : D + 1],
                )
                rec = smalls.tile([128, 1], F32, tag="rec")
                nc.vector.reciprocal(rec, ptr[0:128, D : D + 1])
                nc.vector.tensor_scalar_mul(osb[:, i, :], ptr[0:128, 0:D], rec)
            nc.sync.dma_start(
                out=out[p].rearrange("(t pp) d -> pp t d", pp=128), in_=osb
            )
    nc.compile()
    return nc


_nc_cache = None


def _get_nc():
    global _nc_cache
    if _nc_cache is None:
        _nc_cache = build_nc()
    return _nc_cache


def kernel(q, k, v, mask):
    nc = _get_nc()
    qf = np.ascontiguousarray(q.reshape(B * H, S, D), dtype=np.float32)
    kf = np.ascontiguousarray(k.reshape(B * H, S, D), dtype=np.float32)
    vf = np.ascontiguousarray(v.reshape(B * H, S, D), dtype=np.float32)
    in_maps = [
        {
            "q": qf[i * PAIRS : (i + 1) * PAIRS],
            "k": kf[i * PAIRS : (i + 1) * PAIRS],
            "v": vf[i * PAIRS : (i + 1) * PAIRS],
        }
        for i in range(NCORES)
    ]
    res = run_bass_kernel_spmd(nc, in_maps, core_ids=list(range(NCORES)))
    out = np.concatenate([res.results[i]["out"] for i in range(NCORES)], axis=0)
    return out.reshape(B, H, S, D)


# revision 8
# speedup vs baseline: 1.4039x; 1.4039x over previous
import sys

if "/opt/trn_rl_repo" not in sys.path:
    sys.path.insert(0, "/opt/trn_rl_repo")

from contextlib import ExitStack

import numpy as np

import concourse.bacc as bacc
import concourse.tile as tile
from concourse import mybir
from concourse.bass_utils import run_bass_kernel_spmd
from concourse.masks import make_identity

B, H, S, D = 2, 16, 2048, 64
NCORES = 8
PAIRS = (B * H) // NCORES
NT = S // 128
NSEG = S // 512
F32 = mybir.dt.float32
MM_DT = mybir.dt.float32 if __import__("os").environ.get("MM_F32") else mybir.dt.float32r
SCALE = 0.125
NEG = -1e10


def build_nc():
    nc = bacc.Bacc(None)
    q = nc.declare_dram_parameter("q", [PAIRS, S, D], F32, isOutput=False)
    k = nc.declare_dram_parameter("k", [PAIRS, S, D], F32, isOutput=False)
    v = nc.declare_dram_parameter("v", [PAIRS, S, D], F32, isOutput=False)
    out = nc.declare_dram_parameter("out", [PAIRS, S, D], F32, isOutput=True)

    with tile.TileContext(nc) as tc, ExitStack() as ctx:
        consts = ctx.enter_context(tc.tile_pool(name="consts", bufs=1))
        stage = ctx.enter_context(tc.tile_pool(name="stage", bufs=2))
        qtp = ctx.enter_context(tc.tile_pool(name="qt", bufs=2))
        ktp = ctx.enter_context(tc.tile_pool(name="kt", bufs=2))
        vpp = ctx.enter_context(tc.tile_pool(name="vp", bufs=2))
        ptp = ctx.enter_context(tc.tile_pool(name="pt", bufs=2))
        accs_p = ctx.enter_context(tc.tile_pool(name="accs", bufs=2))
        outp = ctx.enter_context(tc.tile_pool(name="outsb", bufs=2))
        smalls = ctx.enter_context(tc.tile_pool(name="smalls", bufs=4))
        ps_scores = ctx.enter_context(
            tc.tile_pool(name="ps_scores", bufs=2, space="PSUM")
        )
        ps_acc = ctx.enter_context(tc.tile_pool(name="ps_acc", bufs=1, space="PSUM"))
        ps_tr = ctx.enter_context(tc.tile_pool(name="ps_tr", bufs=2, space="PSUM"))

        ident = consts.tile([128, 128], F32)
        make_identity(nc, ident)
        trimask = consts.tile([128, 128], F32)
        nc.gpsimd.memset(trimask, 0.0)
        nc.gpsimd.affine_select(
            out=trimask,
            in_=trimask,
            compare_op=mybir.AluOpType.is_ge,
            fill=NEG,
            base=0,
            pattern=[[1, 128]],
            channel_multiplier=-1,
        )

        for p in range(PAIRS):
            qstg = stage.tile([128, NT, D], F32, tag="qstg")
            kstg = stage.tile([128, NT, D], F32, tag="kstg")
            nc.sync.dma_start(
                out=qstg, in_=q[p].rearrange("(t pp) d -> pp t d", pp=128)
            )
            nc.sync.dma_start(
                out=kstg, in_=k[p].rearrange("(t pp) d -> pp t d", pp=128)
            )
            vstg = stage.tile([128, NT, D + 1], F32, tag="vstg")
            nc.sync.dma_start(
                out=vstg[:, :, 0:D], in_=v[p].rearrange("(t pp) d -> pp t d", pp=128)
            )
            nc.vector.memset(vstg[:, :, D : D + 1], 1.0)
            vp_t = vpp.tile([128, NT, D + 1], MM_DT, tag="vp")
            nc.vector.tensor_copy(vp_t, vstg)

            qt = qtp.tile([64, S], MM_DT, tag="qt")
            kt = ktp.tile([64, S], MM_DT, tag="kt")
            for j in range(NT):
                pst = ps_tr.tile([128, 128], F32, tag="tr")
                nc.tensor.transpose(pst[0:64, 0:128], qstg[:, j, :], ident)
                nc.vector.tensor_copy(qt[:, j * 128 : (j + 1) * 128], pst[0:64, 0:128])
                pst2 = ps_tr.tile([128, 128], F32, tag="tr")
                nc.tensor.transpose(pst2[0:64, 0:128], kstg[:, j, :], ident)
                nc.vector.tensor_copy(kt[:, j * 128 : (j + 1) * 128], pst2[0:64, 0:128])

            acc = ps_acc.tile([128, S], F32, tag="acc")
            for j in range(NT):
                q0 = j * 128
                rem = S - q0
                ptt = ptp.tile([128, S], MM_DT, tag="pt")
                off = 0
                while off < rem:
                    w = min(512, rem - off)
                    ps = ps_scores.tile([128, 512], F32, tag="scores")
                    nc.tensor.matmul(
                        ps[:, 0:w],
                        kt[:, q0 : q0 + 128],
                        qt[:, q0 + off : q0 + off + w],
                        start=True,
                        stop=True,
                    )
                    if off == 0:
                        nc.vector.tensor_add(ps[:, 0:128], ps[:, 0:128], trimask)
                    nc.scalar.activation(
                        ptt[:, off : off + w],
                        ps[:, 0:w],
                        mybir.ActivationFunctionType.Exp,
                        scale=SCALE,
                    )
                    off += w
                for s in range(j // 4, NSEG):
                    gstart = max(512 * s, q0)
                    w2 = 512 * (s + 1) - gstart
                    loc = gstart - q0
                    nc.tensor.matmul(
                        acc[0 : D + 1, gstart : gstart + w2],
                        vp_t[:, j, :],
                        ptt[:, loc : loc + w2],
                        start=(j == 0),
                        stop=(j == 4 * s + 3),
                    )

            accs = accs_p.tile([D + 1, S], F32, tag="accs")
            nc.vector.tensor_copy(accs, acc[0 : D + 1, :])
            osb = outp.tile([128, NT, D], F32, tag="osb")
            for i in range(NT):
                ptr = ps_tr.tile([128, 128], F32, tag="tr")
                nc.tensor.transpose(
                    ptr[0:128, 0 : D + 1],
                    accs[:, i * 128 : (i + 1) * 128],
                    ident[0 : D + 1, 0 : D + 1],
                )
                rec = smalls.tile([128, 1], F32, tag="rec")
                nc.vector.reciprocal(rec, ptr[0:128, D : D + 1])
                nc.vector.tensor_scalar_mul(osb[:, i, :], ptr[0:128, 0:D], rec)
            nc.sync.dma_start(
                out=out[p].rearrange("(t pp) d -> pp t d", pp=128), in_=osb
            )
    nc.compile()
    return nc


_nc_cache = None


def _get_nc():
    global _nc_cache
    if _nc_cache is None:
        _nc_cache = build_nc()
    return _nc_cache


def kernel(q, k, v, mask):
    nc = _get_nc()
    qf = np.ascontiguousarray(q.reshape(B * H, S, D), dtype=np.float32)
    kf = np.ascontiguousarray(k.reshape(B * H, S, D), dtype=np.float32)
    vf = np.ascontiguousarray(v.reshape(B * H, S, D), dtype=np.float32)
    in_maps = [
        {
            "q": qf[i * PAIRS : (i + 1) * PAIRS],
            "k": kf[i * PAIRS : (i + 1) * PAIRS],
            "v": vf[i * PAIRS : (i + 1) * PAIRS],
        }
        for i in range(NCORES)
    ]
    res = run_bass_kernel_spmd(nc, in_maps, core_ids=list(range(NCORES)))
    out = np.concatenate([res.results[i]["out"] for i in range(NCORES)], axis=0)
    return out.reshape(B, H, S, D)


# revision 11
# speedup vs baseline: 2.8696x; 2.0440x over previous
import sys

if "/opt/trn_rl_repo" not in sys.path:
    sys.path.insert(0, "/opt/trn_rl_repo")

from contextlib import ExitStack

import numpy as np

import concourse.bacc as bacc
import concourse.tile as tile
from concourse import mybir
from concourse.bass_utils import run_bass_kernel_spmd

B, H, S, D = 2, 16, 2048, 64
NCORES = 8
PAIRS = (B * H) // NCORES
NT = S // 128
F32 = mybir.dt.float32
BF16 = mybir.dt.bfloat16
SCALE = 0.125


def build_nc():
    nc = bacc.Bacc(None)
    qT = nc.declare_dram_parameter("qT", [PAIRS, D, S], F32, isOutput=False)
    kT = nc.declare_dram_parameter("kT", [PAIRS, D, S], F32, isOutput=False)
    v = nc.declare_dram_parameter("v", [PAIRS, S, D], F32, isOutput=False)
    out = nc.declare_dram_parameter("out", [PAIRS, S, D], F32, isOutput=True)

    with tile.TileContext(nc) as tc, ExitStack() as ctx:
        consts = ctx.enter_context(tc.tile_pool(name="consts", bufs=1))
        stage = ctx.enter_context(tc.tile_pool(name="stage", bufs=2))
        qtp = ctx.enter_context(tc.tile_pool(name="qt", bufs=2))
        ktp = ctx.enter_context(tc.tile_pool(name="kt", bufs=2))
        vpp = ctx.enter_context(tc.tile_pool(name="vp", bufs=2))
        ptp = ctx.enter_context(tc.tile_pool(name="pt", bufs=3))
        outp = ctx.enter_context(tc.tile_pool(name="outsb", bufs=2))
        smalls = ctx.enter_context(tc.tile_pool(name="smalls", bufs=4))
        ps_scores = ctx.enter_context(
            tc.tile_pool(name="ps_scores", bufs=2, space="PSUM")
        )
        ps_acc = ctx.enter_context(tc.tile_pool(name="ps_acc", bufs=1, space="PSUM"))

        tri01 = consts.tile([128, 128], BF16)
        nc.gpsimd.memset(tri01, 1.0)
        nc.gpsimd.affine_select(
            out=tri01,
            in_=tri01,
            compare_op=mybir.AluOpType.is_ge,
            fill=0.0,
            base=0,
            pattern=[[1, 128]],
            channel_multiplier=-1,
        )

        for p in range(PAIRS):
            qts = stage.tile([D, S], F32, tag="qts")
            kts = stage.tile([D, S], F32, tag="kts")
            nc.sync.dma_start(out=qts, in_=qT[p])
            nc.sync.dma_start(out=kts, in_=kT[p])
            vstg = stage.tile([128, NT, D + 1], F32, tag="vstg")
            nc.sync.dma_start(
                out=vstg[:, :, 0:D], in_=v[p].rearrange("(t pp) d -> pp t d", pp=128)
            )
            nc.vector.memset(vstg[:, :, D : D + 1], 1.0)

            qt = qtp.tile([D, S], BF16, tag="qt")
            kt = ktp.tile([D, S], BF16, tag="kt")
            vp_t = vpp.tile([128, NT, D + 1], BF16, tag="vp")
            nc.vector.tensor_copy(qt, qts)
            nc.vector.tensor_copy(kt, kts)
            nc.vector.tensor_copy(vp_t, vstg)

            acc = ps_acc.tile([128, NT, 128], F32, tag="acc")

            def emit_pv(j, ptt):
                for i in range(j, NT):
                    nc.tensor.matmul(
                        acc[:, i, 0 : D + 1],
                        ptt[:, (i - j) * 128 : (i - j + 1) * 128],
                        vp_t[:, j, :],
                        start=(j == 0 and i % 4 == 0),
                        stop=(j == i),
                    )

            prev = None
            for j in range(NT):
                q0 = j * 128
                rem = S - q0
                ptt = ptp.tile([128, S], BF16, tag="pt")
                off = 0
                while off < rem:
                    w = min(1024, rem - off)
                    ps = ps_scores.tile([128, 1024], F32, tag="scores")
                    w0 = min(512, w)
                    nc.tensor.matmul(
                        ps[:, 0:w0],
                        kt[:, q0 : q0 + 128],
                        qt[:, q0 + off : q0 + off + w0],
                        start=True,
                        stop=True,
                    )
                    if w > 512:
                        nc.tensor.matmul(
                            ps[:, 512:w],
                            kt[:, q0 : q0 + 128],
                            qt[:, q0 + off + 512 : q0 + off + w],
                            start=True,
                            stop=True,
                        )
                    nc.scalar.activation(
                        ptt[:, off : off + w],
                        ps[:, 0:w],
                        mybir.ActivationFunctionType.Exp,
                        scale=SCALE,
                    )
                    off += w
                nc.vector.tensor_mul(ptt[:, 0:128], ptt[:, 0:128], tri01)
                if prev is not None:
                    emit_pv(prev[0], prev[1])
                prev = (j, ptt)
            emit_pv(prev[0], prev[1])

            osb = outp.tile([128, NT, D], F32, tag="osb")
            for i in range(NT):
                rec = smalls.tile([128, 1], F32, tag="rec")
                nc.vector.reciprocal(rec, acc[:, i, D : D + 1])
                nc.vector.tensor_scalar_mul(osb[:, i, :], acc[:, i, 0:D], rec)
            nc.sync.dma_start(
                out=out[p].rearrange("(t pp) d -> pp t d", pp=128), in_=osb
            )
    nc.compile()
    return nc


_nc_cache = None


def _get_nc():
    global _nc_cache
    if _nc_cache is None:
        _nc_cache = build_nc()
    return _nc_cache


def kernel(q, k, v, mask):
    nc = _get_nc()
    qf = np.asarray(q, dtype=np.float32).reshape(B * H, S, D)
    kf = np.asarray(k, dtype=np.float32).reshape(B * H, S, D)
    vf = np.ascontiguousarray(np.asarray(v, dtype=np.float32).reshape(B * H, S, D))
    qTf = np.ascontiguousarray(qf.transpose(0, 2, 1))
    kTf = np.ascontiguousarray(kf.transpose(0, 2, 1))
    in_maps = [
        {
            "qT": qTf[i * PAIRS : (i + 1) * PAIRS],
            "kT": kTf[i * PAIRS : (i + 1) * PAIRS],
            "v": vf[i * PAIRS : (i + 1) * PAIRS],
        }
        for i in range(NCORES)
    ]
    res = run_bass_kernel_spmd(nc, in_maps, core_ids=list(range(NCORES)))
    out = np.concatenate([res.results[i]["out"] for i in range(NCORES)], axis=0)
    return out.reshape(B, H, S, D)


# revision 14
# speedup vs baseline: 3.1140x; 1.0852x over previous
import sys

if "/opt/trn_rl_repo" not in sys.path:
    sys.path.insert(0, "/opt/trn_rl_repo")

from contextlib import ExitStack

import numpy as np

import concourse.bass as bass
import concourse.bacc as bacc
import concourse.tile as tile
from concourse import mybir
from concourse.bass_utils import run_bass_kernel_spmd

B, H, S, D = 2, 16, 2048, 64
NCORES = 8
PAIRS = (B * H) // NCORES
NT = S // 128
F32 = mybir.dt.float32
BF16 = mybir.dt.bfloat16
SCALE = 0.125


def build_nc():
    nc = bacc.Bacc(None)
    qT = nc.declare_dram_parameter("qT", [PAIRS, D, S], F32, isOutput=False)
    kT = nc.declare_dram_parameter("kT", [PAIRS, D, S], F32, isOutput=False)
    v = nc.declare_dram_parameter("v", [PAIRS, S, D], F32, isOutput=False)
    out = nc.declare_dram_parameter("out", [PAIRS, S, D], F32, isOutput=True)

    with tile.TileContext(nc) as tc, ExitStack() as ctx:
        consts = ctx.enter_context(tc.tile_pool(name="consts", bufs=1))
        stage = ctx.enter_context(tc.tile_pool(name="stage", bufs=2))
        qtp = ctx.enter_context(tc.tile_pool(name="qt", bufs=2))
        ktp = ctx.enter_context(tc.tile_pool(name="kt", bufs=2))
        vpp = ctx.enter_context(tc.tile_pool(name="vp", bufs=2))
        ptp = ctx.enter_context(tc.tile_pool(name="pt", bufs=3))
        outp = ctx.enter_context(tc.tile_pool(name="outsb", bufs=2))
        smalls = ctx.enter_context(tc.tile_pool(name="smalls", bufs=4))
        ps_scores = ctx.enter_context(
            tc.tile_pool(name="ps_scores", bufs=2, space="PSUM")
        )
        ps_acc = ctx.enter_context(tc.tile_pool(name="ps_acc", bufs=1, space="PSUM"))

        tri01 = consts.tile([128, 128], BF16)
        nc.gpsimd.memset(tri01, 1.0)
        nc.gpsimd.affine_select(
            out=tri01,
            in_=tri01,
            compare_op=mybir.AluOpType.is_ge,
            fill=0.0,
            base=0,
            pattern=[[1, 128]],
            channel_multiplier=-1,
        )

        for p in range(PAIRS):
            qts = stage.tile([D, S], F32, tag="qts")
            kts = stage.tile([D, S], F32, tag="kts")
            nc.sync.dma_start(out=qts, in_=qT[p])
            nc.sync.dma_start(out=kts, in_=kT[p])
            vstg = stage.tile([128, NT, D + 1], F32, tag="vstg")
            nc.sync.dma_start(
                out=vstg[:, :, 0:D], in_=v[p].rearrange("(t pp) d -> pp t d", pp=128)
            )
            nc.vector.memset(vstg[:, :, D : D + 1], 1.0)

            qt = qtp.tile([D, S], BF16, tag="qt")
            kt = ktp.tile([D, S], BF16, tag="kt")
            vp_t = vpp.tile([128, NT, D + 1], BF16, tag="vp")
            nc.vector.tensor_copy(qt, qts)
            nc.vector.tensor_copy(kt, kts)
            nc.vector.tensor_copy(vp_t, vstg)

            acc = ps_acc.tile([128, NT, 128], F32, tag="acc")

            def emit_pv(j, ptt):
                for i in range(j, NT):
                    nc.tensor.matmul(
                        acc[:, i, 0 : D + 1],
                        ptt[:, (i - j) * 128 : (i - j + 1) * 128],
                        vp_t[:, j, :],
                        start=(j == 0 and i % 4 == 0),
                        stop=(j == i),
                    )

            prev = None
            for j in range(NT):
                q0 = j * 128
                rem = S - q0
                ptt = ptp.tile([128, S], BF16, tag="pt")
                off = 0
                while off < rem:
                    w = min(1024, rem - off)
                    ps = ps_scores.tile([128, 1024], F32, tag="scores")
                    w0 = min(512, w)
                    nc.tensor.matmul(
                        ps[:, 0:w0],
                        kt[:, q0 : q0 + 128],
                        qt[:, q0 + off : q0 + off + w0],
                        start=True,
                        stop=True,
                    )
                    if w > 512:
                        nc.tensor.matmul(
                            ps[:, 512:w],
                            kt[:, q0 : q0 + 128],
                            qt[:, q0 + off + 512 : q0 + off + w],
                            start=True,
                            stop=True,
                        )
                    nc.scalar.activation(
                        ptt[:, off : off + w],
                        ps[:, 0:w],
                        mybir.ActivationFunctionType.Exp,
                        scale=SCALE,
                    )
                    off += w
                nc.vector.tensor_mul(ptt[:, 0:128], ptt[:, 0:128], tri01)
                if prev is not None:
                    emit_pv(prev[0], prev[1])
                prev = (j, ptt)
            emit_pv(prev[0], prev[1])

            accs = outp.tile([128, NT, D + 1], F32, tag="accs")
            nc.vector.tensor_copy(accs, acc[:, :, 0 : D + 1])
            rec16 = smalls.tile([128, NT], F32, tag="rec")
            nc.vector.reciprocal(rec16, accs[:, :, D])
            osb = outp.tile([128, NT, D], F32, tag="osb")
            r16 = rec16[:, :]
            rec_bcast = bass.AP(
                tensor=r16.tensor,
                offset=r16.offset,
                ap=[r16.ap[0], r16.ap[1], [0, D]],
            )
            nc.vector.tensor_mul(osb, accs[:, :, 0:D], rec_bcast)
            nc.sync.dma_start(
                out=out[p].rearrange("(t pp) d -> pp t d", pp=128), in_=osb
            )
    nc.compile()
    return nc


_nc_cache = None


def _get_nc():
    global _nc_cache
    if _nc_cache is None:
        _nc_cache = build_nc()
    return _nc_cache


def kernel(q, k, v, mask):
    nc = _get_nc()
    qf = np.asarray(q, dtype=np.float32).reshape(B * H, S, D)
    kf = np.asarray(k, dtype=np.float32).reshape(B * H, S, D)
    vf = np.ascontiguousarray(np.asarray(v, dtype=np.float32).reshape(B * H, S, D))
    qTf = np.ascontiguousarray(qf.transpose(0, 2, 1))
    kTf = np.ascontiguousarray(kf.transpose(0, 2, 1))
    in_maps = [
        {
            "qT": qTf[i * PAIRS : (i + 1) * PAIRS],
            "kT": kTf[i * PAIRS : (i + 1) * PAIRS],
            "v": vf[i * PAIRS : (i + 1) * PAIRS],
        }
        for i in range(NCORES)
    ]
    res = run_bass_kernel_spmd(nc, in_maps, core_ids=list(range(NCORES)))
    out = np.concatenate([res.results[i]["out"] for i in range(NCORES)], axis=0)
    return out.reshape(B, H, S, D)


# revision 16
# speedup vs baseline: 3.2608x; 1.0472x over previous
import sys

if "/opt/trn_rl_repo" not in sys.path:
    sys.path.insert(0, "/opt/trn_rl_repo")

from contextlib import ExitStack

import ml_dtypes
import numpy as np

import concourse.bass as bass
import concourse.bacc as bacc
import concourse.tile as tile
from concourse import mybir
from concourse.bass_utils import run_bass_kernel_spmd

B, H, S, D = 2, 16, 2048, 64
NCORES = 8
PAIRS = (B * H) // NCORES
NT = S // 128
F32 = mybir.dt.float32
BF16 = mybir.dt.bfloat16
SCALE = 0.125


def build_nc():
    nc = bacc.Bacc(None)
    qT = nc.declare_dram_parameter("qT", [PAIRS, D, S], BF16, isOutput=False)
    kT = nc.declare_dram_parameter("kT", [PAIRS, D, S], BF16, isOutput=False)
    v = nc.declare_dram_parameter("v", [PAIRS, S, D], BF16, isOutput=False)
    out = nc.declare_dram_parameter("out", [PAIRS, S, D], F32, isOutput=True)

    with tile.TileContext(nc) as tc, ExitStack() as ctx:
        consts = ctx.enter_context(tc.tile_pool(name="consts", bufs=1))
        stage = ctx.enter_context(tc.tile_pool(name="stage", bufs=2))
        qtp = ctx.enter_context(tc.tile_pool(name="qt", bufs=2))
        ktp = ctx.enter_context(tc.tile_pool(name="kt", bufs=2))
        vpp = ctx.enter_context(tc.tile_pool(name="vp", bufs=2))
        ptp = ctx.enter_context(tc.tile_pool(name="pt", bufs=3))
        outp = ctx.enter_context(tc.tile_pool(name="outsb", bufs=2))
        smalls = ctx.enter_context(tc.tile_pool(name="smalls", bufs=4))
        ps_scores = ctx.enter_context(
            tc.tile_pool(name="ps_scores", bufs=2, space="PSUM")
        )
        ps_acc = ctx.enter_context(tc.tile_pool(name="ps_acc", bufs=1, space="PSUM"))

        tri01 = consts.tile([128, 128], BF16)
        nc.gpsimd.memset(tri01, 1.0)
        nc.gpsimd.affine_select(
            out=tri01,
            in_=tri01,
            compare_op=mybir.AluOpType.is_ge,
            fill=0.0,
            base=0,
            pattern=[[1, 128]],
            channel_multiplier=-1,
        )

        for p in range(PAIRS):
            qt = qtp.tile([D, S], BF16, tag="qt")
            kt = ktp.tile([D, S], BF16, tag="kt")
            vp_t = vpp.tile([128, NT, D + 1], BF16, tag="vp")
            nc.sync.dma_start(out=qt, in_=qT[p])
            nc.sync.dma_start(out=kt, in_=kT[p])
            nc.sync.dma_start(
                out=vp_t[:, :, 0:D],
                in_=v[p].rearrange("(t pp) d -> pp t d", pp=128),
            )
            nc.vector.memset(vp_t[:, :, D : D + 1], 1.0)

            acc = ps_acc.tile([128, NT, 128], F32, tag="acc")

            def emit_pv(j, ptt):
                for i in range(j, NT):
                    nc.tensor.matmul(
                        acc[:, i, 0 : D + 1],
                        ptt[:, (i - j) * 128 : (i - j + 1) * 128],
                        vp_t[:, j, :],
                        start=(j == 0 and i % 4 == 0),
                        stop=(j == i),
                    )

            out_r = out[p].rearrange("(t pp) d -> pp t d", pp=128)

            def emit_finish(g):
                g0 = 4 * g
                accs = outp.tile([128, 4, D + 1], F32, tag="accs")
                nc.vector.tensor_copy(accs, acc[:, g0 : g0 + 4, 0 : D + 1])
                rec4 = smalls.tile([128, 4], F32, tag="rec")
                nc.vector.reciprocal(rec4, accs[:, :, D])
                osb = outp.tile([128, 4, D], F32, tag="osb")
                r4 = rec4[:, :]
                rec_bcast = bass.AP(
                    tensor=r4.tensor,
                    offset=r4.offset,
                    ap=[r4.ap[0], r4.ap[1], [0, D]],
                )
                nc.vector.tensor_mul(osb, accs[:, :, 0:D], rec_bcast)
                nc.sync.dma_start(out=out_r[:, g0 : g0 + 4, :], in_=osb)

            prev = None
            for j in range(NT):
                q0 = j * 128
                rem = S - q0
                ptt = ptp.tile([128, S], BF16, tag="pt")
                off = 0
                while off < rem:
                    w = min(1024, rem - off)
                    ps = ps_scores.tile([128, 1024], F32, tag="scores")
                    w0 = min(512, w)
                    nc.tensor.matmul(
                        ps[:, 0:w0],
                        kt[:, q0 : q0 + 128],
                        qt[:, q0 + off : q0 + off + w0],
                        start=True,
                        stop=True,
                    )
                    if w > 512:
                        nc.tensor.matmul(
                            ps[:, 512:w],
                            kt[:, q0 : q0 + 128],
                            qt[:, q0 + off + 512 : q0 + off + w],
                            start=True,
                            stop=True,
                        )
                    nc.scalar.activation(
                        ptt[:, off : off + w],
                        ps[:, 0:w],
                        mybir.ActivationFunctionType.Exp,
                        scale=SCALE,
                    )
                    off += w
                nc.vector.tensor_mul(ptt[:, 0:128], ptt[:, 0:128], tri01)
                if prev is not None:
                    emit_pv(prev[0], prev[1])
                    if prev[0] % 4 == 3:
                        emit_finish(prev[0] // 4)
                prev = (j, ptt)
            emit_pv(prev[0], prev[1])
            emit_finish(NT // 4 - 1)
    nc.compile()
    return nc


_nc_cache = None


def _get_nc():
    global _nc_cache
    if _nc_cache is None:
        _nc_cache = build_nc()
    return _nc_cache


def kernel(q, k, v, mask):
    nc = _get_nc()
    bf = ml_dtypes.bfloat16
    qf = np.asarray(q, dtype=np.float32).reshape(B * H, S, D)
    kf = np.asarray(k, dtype=np.float32).reshape(B * H, S, D)
    vf = np.ascontiguousarray(
        np.asarray(v, dtype=np.float32).reshape(B * H, S, D).astype(bf)
    )
    qTf = np.ascontiguousarray(qf.transpose(0, 2, 1).astype(bf))
    kTf = np.ascontiguousarray(kf.transpose(0, 2, 1).astype(bf))
    in_maps = [
        {
            "qT": qTf[i * PAIRS : (i + 1) * PAIRS],
            "kT": kTf[i * PAIRS : (i + 1) * PAIRS],
            "v": vf[i * PAIRS : (i + 1) * PAIRS],
        }
        for i in range(NCORES)
    ]
    res = run_bass_kernel_spmd(nc, in_maps, core_ids=list(range(NCORES)))
    out = np.concatenate([res.results[i]["out"] for i in range(NCORES)], axis=0)
    return out.reshape(B, H, S, D)


# revision 17
# speedup vs baseline: 3.3823x; 1.0373x over previous
import sys

if "/opt/trn_rl_repo" not in sys.path:
    sys.path.insert(0, "/opt/trn_rl_repo")

from contextlib import ExitStack

import ml_dtypes
import numpy as np

import concourse.bass as bass
import concourse.bacc as bacc
import concourse.tile as tile
from concourse import mybir
from concourse.bass_utils import run_bass_kernel_spmd

B, H, S, D = 2, 16, 2048, 64
NCORES = 8
PAIRS = (B * H) // NCORES
NT = S // 128
F32 = mybir.dt.float32
BF16 = mybir.dt.bfloat16
SCALE = 0.125


def build_nc():
    nc = bacc.Bacc(None)
    qT = nc.declare_dram_parameter("qT", [PAIRS, D, S], BF16, isOutput=False)
    kT = nc.declare_dram_parameter("kT", [PAIRS, D, S], BF16, isOutput=False)
    v = nc.declare_dram_parameter("v", [PAIRS, S, D], BF16, isOutput=False)
    out = nc.declare_dram_parameter("out", [PAIRS, S, D], F32, isOutput=True)

    with tile.TileContext(nc) as tc, ExitStack() as ctx:
        consts = ctx.enter_context(tc.tile_pool(name="consts", bufs=1))
        stage = ctx.enter_context(tc.tile_pool(name="stage", bufs=2))
        qtp = ctx.enter_context(tc.tile_pool(name="qt", bufs=2))
        ktp = ctx.enter_context(tc.tile_pool(name="kt", bufs=2))
        vpp = ctx.enter_context(tc.tile_pool(name="vp", bufs=2))
        ptp = ctx.enter_context(tc.tile_pool(name="pt", bufs=3))
        outp = ctx.enter_context(tc.tile_pool(name="outsb", bufs=2))
        smalls = ctx.enter_context(tc.tile_pool(name="smalls", bufs=4))
        ps_scores = ctx.enter_context(
            tc.tile_pool(name="ps_scores", bufs=2, space="PSUM")
        )
        ps_acc = ctx.enter_context(tc.tile_pool(name="ps_acc", bufs=1, space="PSUM"))

        tri01 = consts.tile([128, 128], BF16)
        nc.gpsimd.memset(tri01, 1.0)
        nc.gpsimd.affine_select(
            out=tri01,
            in_=tri01,
            compare_op=mybir.AluOpType.is_ge,
            fill=0.0,
            base=0,
            pattern=[[1, 128]],
            channel_multiplier=-1,
        )

        wq = ps_scores.tile([128, 512], F32, tag="scores")
        t01 = tri01[:, :]
        tri_rep = bass.AP(
            tensor=t01.tensor,
            offset=t01.offset,
            ap=[t01.ap[0], [0, 4], t01.ap[1]],
        )
        for _ in range(24):
            nc.tensor.matmul(wq[:, 0:512], tri01, tri_rep, start=True, stop=True)

        for p in range(PAIRS):
            qt = qtp.tile([D, S], BF16, tag="qt")
            kt = ktp.tile([D, S], BF16, tag="kt")
            vp_t = vpp.tile([128, NT, D + 1], BF16, tag="vp")
            nc.sync.dma_start(out=qt, in_=qT[p])
            nc.sync.dma_start(out=kt, in_=kT[p])
            nc.sync.dma_start(
                out=vp_t[:, :, 0:D],
                in_=v[p].rearrange("(t pp) d -> pp t d", pp=128),
            )
            nc.vector.memset(vp_t[:, :, D : D + 1], 1.0)

            acc = ps_acc.tile([128, NT, 128], F32, tag="acc")

            def emit_pv(j, ptt):
                for i in range(j, NT):
                    nc.tensor.matmul(
                        acc[:, i, 0 : D + 1],
                        ptt[:, (i - j) * 128 : (i - j + 1) * 128],
                        vp_t[:, j, :],
                        start=(j == 0 and i % 4 == 0),
                        stop=(j == i),
                    )

            out_r = out[p].rearrange("(t pp) d -> pp t d", pp=128)

            def emit_finish(g):
                g0 = 4 * g
                accs = outp.tile([128, 4, D + 1], F32, tag="accs")
                nc.vector.tensor_copy(accs, acc[:, g0 : g0 + 4, 0 : D + 1])
                rec4 = smalls.tile([128, 4], F32, tag="rec")
                nc.vector.reciprocal(rec4, accs[:, :, D])
                osb = outp.tile([128, 4, D], F32, tag="osb")
                r4 = rec4[:, :]
                rec_bcast = bass.AP(
                    tensor=r4.tensor,
                    offset=r4.offset,
                    ap=[r4.ap[0], r4.ap[1], [0, D]],
                )
                nc.vector.tensor_mul(osb, accs[:, :, 0:D], rec_bcast)
                nc.sync.dma_start(out=out_r[:, g0 : g0 + 4, :], in_=osb)

            prev = None
            for j in range(NT):
                q0 = j * 128
                rem = S - q0
                if prev is not None:
                    emit_pv(prev[0], prev[1])
                    if prev[0] % 4 == 3:
                        emit_finish(prev[0] // 4)
                ptt = ptp.tile([128, S], BF16, tag="pt")
                off = 0
                while off < rem:
                    w = min(1024, rem - off)
                    ps = ps_scores.tile([128, 1024], F32, tag="scores")
                    w0 = min(512, w)
                    nc.tensor.matmul(
                        ps[:, 0:w0],
                        kt[:, q0 : q0 + 128],
                        qt[:, q0 + off : q0 + off + w0],
                        start=True,
                        stop=True,
                    )
                    if w > 512:
                        nc.tensor.matmul(
                            ps[:, 512:w],
                            kt[:, q0 : q0 + 128],
                            qt[:, q0 + off + 512 : q0 + off + w],
                            start=True,
                            stop=True,
                        )
                    nc.scalar.activation(
                        ptt[:, off : off + w],
                        ps[:, 0:w],
                        mybir.ActivationFunctionType.Exp,
                        scale=SCALE,
                    )
                    off += w
                nc.vector.tensor_mul(ptt[:, 0:128], ptt[:, 0:128], tri01)
                prev = (j, ptt)
            emit_pv(prev[0], prev[1])
            emit_finish(NT // 4 - 1)
    nc.compile()
    return nc


_nc_cache = None


def _get_nc():
    global _nc_cache
    if _nc_cache is None:
        _nc_cache = build_nc()
    return _nc_cache


def kernel(q, k, v, mask):
    nc = _get_nc()
    bf = ml_dtypes.bfloat16
    qf = np.asarray(q, dtype=np.float32).reshape(B * H, S, D)
    kf = np.asarray(k, dtype=np.float32).reshape(B * H, S, D)
    vf = np.ascontiguousarray(
        np.asarray(v, dtype=np.float32).reshape(B * H, S, D).astype(bf)
    )
    qTf = np.ascontiguousarray(qf.transpose(0, 2, 1).astype(bf))
    kTf = np.ascontiguousarray(kf.transpose(0, 2, 1).astype(bf))
    in_maps = [
        {
            "qT": qTf[i * PAIRS : (i + 1) * PAIRS],
            "kT": kTf[i * PAIRS : (i + 1) * PAIRS],
            "v": vf[i * PAIRS : (i + 1) * PAIRS],
        }
        for i in range(NCORES)
    ]
    res = run_bass_kernel_spmd(nc, in_maps, core_ids=list(range(NCORES)))
    out = np.concatenate([res.results[i]["out"] for i in range(NCORES)], axis=0)
    return out.reshape(B, H, S, D)
